# revision 21
# baseline (speedup 1.0000x reference)
"""Distributed Trainium2 kernel for nn_Curating_of_attention_mask.

Math: batch (3,1280,1280) -> 6400 patches of 16x16 -> per-patch 3x3 channel
gram -> pairwise squared-distance matrix (6400,6400) -> global min/max
normalize -> (1,6400,6400).

Simplifications vs the reference:
 - (d - min)/(max - min) is invariant to positive rescaling, so constant
   divisors are dropped.
 - dist[i,i] = 0 and dist >= 0, so the global min is 0 (up to ~1e-7 fp
   noise): only the max must be computed/reduced; out = raw / max.
 - Grams are centered by the expected gram of unit-variance noise (256 on
   the diagonal), shrinking magnitudes ~30x and every rounding error.
 - With the 6 unique centered gram entries m and q = sum(diag^2)+2*sum(off^2),
   v = [m/16, 1, q/256] and u = [-2*m_d/16, -4*m_o/16, q/256, 1] give
   raw[i,j] = u_i . v_j = (q_i + q_j - 2<gram_i,gram_j>)/256 >= 0.
   The /16 prescale keeps everything in fp16 range so the big matmuls run
   at 1 cycle/row.

Layout: input slab loads CONTIGUOUSLY as row-major [row, col] tiles
([128,1280] + [32,1280] per channel); per-patch pixel sums happen as
(a) elementwise products + grouped 16-column reduces on ACT/GpSimd/DVE,
(b) a block-ones matmul on the PE to sum the 16 rows of each patch row.

Sharding: patch dim across 8 cores (core k owns image rows [160k,160k+160)
= patches [800k,800k+800)).  Each core builds u/v for its 800 patches,
AllGathers v (fp16), computes its [800,6400] slice of raw twice (pass 1
reduces the max straight out of PSUM, pass 2 applies 1/max during the
PSUM->SBUF drain and streams the result to HBM).  A dummy AllGather at
t=0 warms the collective stream so the real gather isn't serialized
behind the first-collective barrier.
"""

import numpy as np

import concourse.bass as bass
import concourse.bass_isa as bass_isa
import concourse.mybir as mybir
import concourse.tile as tile
from concourse.bass_utils import run_bass_kernel_spmd

F32 = mybir.dt.float32
FP16 = mybir.dt.float16
AX = mybir.AluOpType
AFT = mybir.ActivationFunctionType

N_CORES = 8
C, H, W = 3, 1280, 1280
PS = 16
HP, WP = H // PS, W // PS            # 80, 80
N = HP * WP                          # 6400
H_LOC = H // N_CORES                 # 160 image rows per core
TP = H_LOC // PS                     # 10 patch-rows per core
N_LOC = TP * WP                      # 800 patches per core
K = 8                                # feature dim of u/v
NEG_BIG = -3.0e38
CENTER = 256.0

# feature order: diagonals first, then off-diagonals
PAIRS = [(0, 0), (1, 1), (2, 2), (0, 1), (0, 2), (1, 2)]

# output row tiles and column chunks (each chunk = one PSUM tile; matmuls
# are issued in <=512-column sub-tiles because a PSUM bank is 512 f32)
M_TILES = [(ms, min(128, N_LOC - ms)) for ms in range(0, N_LOC, 128)]     # 7
CHUNKS = [(cs, min(1024, N - cs)) for cs in range(0, N, 1024)]            # 7
SUB = 512

# phase-A input column chunks
ACH = [(0, 640), (640, 640)]

# walrus in this container accepts at most 1 sync-wait command per
# instruction; Tile's tail drain can carry several.  Split extras onto
# preceding NOPs on the same engine (stream order preserves semantics).
_MAX_WAITS = 1


def _split_sync_waits(nc):
    n_fixed = 0
    for func in nc.m.functions:
        for bb in func.blocks:
            new_insts = []
            for inst in bb.instructions:
                si = inst.sync_info
                if si is not None and si.on_wait and len(si.on_wait) > _MAX_WAITS:
                    waits = list(si.on_wait)
                    keep = waits[-_MAX_WAITS:]
                    extra = waits[:-_MAX_WAITS]
                    chunks = [
                        extra[i : i + _MAX_WAITS]
                        for i in range(0, len(extra), _MAX_WAITS)
                    ]
                    for ci, chunk in enumerate(chunks):
                        nop = mybir.InstNoOp(
                            name=f"{inst.name}-waitsplit-{ci}",
                            engine=inst.engine,
                            ins=[],
                            outs=[],
                            sync_info=mybir.SyncInfo(on_wait=chunk, on_update=[]),
                        )
                        new_insts.append(nop)
                        n_fixed += 1
                    si.on_wait = keep
                new_insts.append(inst)
            bb.instructions[:] = new_insts
    return n_fixed


def _build():
    nc = bass.Bass(num_devices=N_CORES)
    x = nc.dram_tensor("x", [C, H_LOC, W], F32, kind="ExternalInput")
    out = nc.dram_tensor("out", [N_LOC, N], F32, kind="ExternalOutput")
    groups = [list(range(N_CORES))]

    with tile.TileContext(nc, num_cores=N_CORES) as tc:
        with (
            tc.tile_pool(name="dram", bufs=1, space="DRAM") as dpool,
            tc.tile_pool(name="cst", bufs=1) as cst,
            tc.tile_pool(name="obig", bufs=2) as obig,
            tc.tile_pool(name="ps", bufs=1, space="PSUM") as psp,
        ):
            v_dram = dpool.tile([K, N_LOC], FP16, name="v_dram")
            u_dram = dpool.tile([K, N_LOC], FP16, name="u_dram")
            vall = dpool.tile([K * N_CORES, N_LOC], FP16, addr_space="Shared",
                              name="vall")
            cc_wo = dpool.tile([1, 8 * N_CORES], F32, addr_space="Shared",
                               name="cc_wo")
            cc_in = dpool.tile([1, 8], F32, name="cc_in")
            cc_out = dpool.tile([1, 8], F32, addr_space="Shared", name="cc_out")

            # ---- warm the collective stream: dummy 32-byte gather at t=0.
            # The CC cores boot ~21us into the NEFF and the first collective
            # pays a ~30us barrier plus a ~15us first-op penalty; paying all
            # of that on a dummy while phase A runs keeps the real gather at
            # its ~5.5us steady-state cost.
            cc_wi = dpool.tile([1, 8], F32, name="cc_wi")
            w8 = cst.tile([1, 8], F32, name="w8")
            nc.vector.memset(w8[:, :], 0.0)
            nc.sync.dma_start(cc_wi[:, :], w8[:, :])
            nc.gpsimd.collective_compute(
                "AllGather",
                AX.bypass,
                replica_groups=groups,
                ins=[cc_wi.opt()],
                outs=[cc_wo.opt()],
            )

            # constants: patch-row summing matrices (block-ones)
            # S[r, t] = 1 iff r//16 == t, built as 0 <= (r - 16t) < 16
            def block_ones(name, nparts, ncols):
                d = cst.tile([nparts, ncols], F32, name=f"{name}_d")
                nc.gpsimd.iota(d[:, :], pattern=[[-16, ncols]], base=0,
                               channel_multiplier=1,
                               allow_small_or_imprecise_dtypes=True)
                ge = cst.tile([nparts, ncols], F32, name=f"{name}_ge")
                nc.vector.tensor_scalar(out=ge[:, :], in0=d[:, :],
                                        scalar1=0.0, scalar2=None,
                                        op0=AX.is_ge)
                s = cst.tile([nparts, ncols], F32, name=name)
                nc.vector.tensor_scalar(out=s[:, :], in0=d[:, :],
                                        scalar1=16.0, scalar2=None,
                                        op0=AX.is_lt)
                nc.vector.tensor_mul(s[:, :], s[:, :], ge[:, :])
                return s

            S1 = block_ones("S1", 128, K)
            S2 = block_ones("S2", 32, 2)

            racc = cst.tile([128, 56], F32, name="racc")
            nc.vector.memset(racc[:, :], NEG_BIG)
            sc8 = cst.tile([1, 8], F32, name="sc8")
            nc.vector.memset(sc8[:, :], NEG_BIG)
            # pre-stage the AllReduce input padding early (off critical path)
            nc.sync.dma_start(cc_in[:, :], sc8[:, :])

            # identity matrix for the PE transpose in the max fold
            iota2d = cst.tile([128, 128], F32, name="iota2d")
            nc.gpsimd.iota(iota2d[:, :], pattern=[[1, 128]], base=0,
                           channel_multiplier=0,
                           allow_small_or_imprecise_dtypes=True)
            iota_col = cst.tile([128, 1], F32, name="iota_col")
            nc.gpsimd.iota(iota_col[:, :], pattern=[[0, 1]], base=0,
                           channel_multiplier=1,
                           allow_small_or_imprecise_dtypes=True)
            ident = cst.tile([128, 128], F32, name="ident")
            nc.vector.tensor_scalar(
                out=ident[:, :], in0=iota2d[:, :], scalar1=iota_col[:, 0:1],
                scalar2=None, op0=AX.is_equal,
            )

            # ---- phase A: contiguous row-major loads + per-patch grams ----
            # feature-major pcol layout [p, f*80+w] so reduce writes are
            # contiguous (strided writes halve DVE throughput)
            pcol6a = cst.tile([128, 6 * WP], F32, name="pcol6a")
            pcol6b = cst.tile([32, 6 * WP], F32, name="pcol6b")
            pca_r = pcol6a.rearrange("p (f w) -> p f w", w=WP)
            pcb_r = pcol6b.rearrange("p (f w) -> p f w", w=WP)

            with (
                tc.tile_pool(name="xsl", bufs=1) as xsl,
                tc.tile_pool(name="prodp", bufs=3) as prodp,
            ):
                xa = [xsl.tile([128, W], F32, name=f"xa{c}") for c in range(C)]
                xb = [xsl.tile([32, W], F32, name=f"xb{c}") for c in range(C)]
                dmae = [nc.sync, nc.scalar, nc.gpsimd]
                for h, (hs, hw) in enumerate(ACH):
                    sl = slice(hs, hs + hw)
                    for c in range(C):
                        dmae[c].dma_start(xa[c][:, sl], x[c, 0:128, sl])
                        dmae[c].dma_start(xb[c][:, sl], x[c, 128:160, sl])
                    wsl = slice(hs // PS, (hs + hw) // PS)
                    for f, (a, b) in enumerate(PAIRS):
                        pa = prodp.tile([128, 640], F32, name="pa", tag="pa")
                        pb = prodp.tile([32, 640], F32, name="pb", tag="pb")
                        if a == b:
                            nc.scalar.activation(pa[:, :], xa[a][:, sl],
                                                 AFT.Square)
                            nc.scalar.activation(pb[:, :], xb[a][:, sl],
                                                 AFT.Square)
                        else:
                            nc.vector.tensor_mul(pa[:, :], xa[a][:, sl],
                                                 xa[b][:, sl])
                            nc.vector.tensor_mul(pb[:, :], xb[a][:, sl],
                                                 xb[b][:, sl])
                        nc.vector.tensor_reduce(
                            out=pca_r[:, f, wsl],
                            in_=pa.rearrange("p (w b) -> p w b", b=PS),
                            axis=mybir.AxisListType.X,
                            op=AX.add,
                        )
                        nc.vector.tensor_reduce(
                            out=pcb_r[:, f, wsl],
                            in_=pb.rearrange("p (w b) -> p w b", b=PS),
                            axis=mybir.AxisListType.X,
                            op=AX.add,
                        )

            # sum the 16 rows of each patch row on the PE
            psG = psp.tile([128, 1024], F32, name="psG", tag="ps", bufs=4)
            nc.tensor.matmul(psG[0:8, 0 : 6 * WP], lhsT=S1[:, :],
                             rhs=pcol6a[:, :], start=True, stop=True)
            nc.tensor.matmul(psG[32:34, 0 : 6 * WP], lhsT=S2[:, :],
                             rhs=pcol6b[:, :], start=True, stop=True)
            G = cst.tile([TP, 6 * WP], F32, name="G")
            nc.scalar.activation(G[0:8, :], psG[0:8, 0 : 6 * WP], AFT.Copy)
            # partitions 32:34 -> 8:10 need a rebase: ACT copy (32-aligned)
            # into SBUF, then an SBUF->SBUF DMA (DMA rebases freely)
            Gtmp = cst.tile([34, 6 * WP], F32, name="Gtmp")
            nc.scalar.activation(Gtmp[32:34, :], psG[32:34, 0 : 6 * WP],
                                 AFT.Copy)
            nc.sync.dma_start(G[8:10, :], Gtmp[32:34, :])
            G_f = G.rearrange("p (f w) -> p f w", w=WP)

            # center diagonal gram entries, then q = sum(d^2) + 2*sum(o^2)
            nc.vector.tensor_scalar_add(G_f[:, 0:3, :], G_f[:, 0:3, :], -CENTER)
            Gsq = cst.tile([TP, 6 * WP], F32, name="Gsq")
            nc.vector.tensor_mul(Gsq[:, :], G[:, :], G[:, :])
            Gsq_f = Gsq.rearrange("p (f w) -> p f w", w=WP)
            qd = cst.tile([TP, WP], F32, name="qd")
            qo = cst.tile([TP, WP], F32, name="qo")
            nc.vector.tensor_add(qd[:, :], Gsq_f[:, 0, :], Gsq_f[:, 1, :])
            nc.vector.tensor_add(qd[:, :], qd[:, :], Gsq_f[:, 2, :])
            nc.vector.tensor_add(qo[:, :], Gsq_f[:, 3, :], Gsq_f[:, 4, :])
            nc.vector.tensor_add(qo[:, :], qo[:, :], Gsq_f[:, 5, :])
            q = cst.tile([TP, WP], F32, name="q")
            nc.vector.scalar_tensor_tensor(
                out=q[:, :], in0=qo[:, :], scalar=2.0, in1=qd[:, :],
                op0=AX.mult, op1=AX.add,
            )

            # fp16 feature heaps [t, slot*80+w]; slots 0-5 gram, 6/7 one & q
            vheap = cst.tile([TP, K * WP], FP16, name="vheap")
            uheap = cst.tile([TP, K * WP], FP16, name="uheap")
            vh_r = vheap.rearrange("p (s w) -> p s w", w=WP)
            uh_r = uheap.rearrange("p (s w) -> p s w", w=WP)
            nc.scalar.activation(vh_r[:, 0:6, :], G_f[:, 0:6, :], AFT.Copy,
                                 scale=1.0 / 16.0)
            nc.vector.memset(vheap[:, 6 * WP : 7 * WP], 1.0)
            nc.scalar.activation(vheap[:, 7 * WP : 8 * WP], q[:, :], AFT.Copy,
                                 scale=1.0 / 256.0)
            nc.sync.dma_start(
                v_dram.rearrange("s (t w) -> t s w", w=WP), vh_r[:, :, :]
            )

            # ---- all-gather v across cores (fp16) ----
            nc.gpsimd.collective_compute(
                "AllGather",
                AX.bypass,
                replica_groups=groups,
                ins=[v_dram.opt()],
                outs=[vall.opt()],
            )

            nc.scalar.activation(uh_r[:, 0:3, :], G_f[:, 0:3, :], AFT.Copy,
                                 scale=-2.0 / 16.0)
            nc.scalar.activation(uh_r[:, 3:6, :], G_f[:, 3:6, :], AFT.Copy,
                                 scale=-4.0 / 16.0)
            nc.scalar.activation(uheap[:, 6 * WP : 7 * WP], q[:, :], AFT.Copy,
                                 scale=1.0 / 256.0)
            nc.vector.memset(uheap[:, 7 * WP : 8 * WP], 1.0)
            nc.scalar.dma_start(
                u_dram.rearrange("s (t w) -> t s w", w=WP), uh_r[:, :, :]
            )
            lhsT = cst.tile([K, N_LOC], FP16, name="lhsT")
            nc.scalar.dma_start(lhsT[:, :], u_dram[:, :])

            rhs = cst.tile([K, N], FP16, name="rhs")
            nc.sync.dma_start(
                rhs.rearrange("f (r l) -> f r l", l=N_LOC),
                vall.rearrange("(r f) l -> f r l", f=K),
            )

            def emit_mms(ms, mh, cs, cw, name):
                ps_t = psp.tile([128, 1024], F32, name=name, tag="ps", bufs=4)
                for ss in range(cs, cs + cw, SUB):
                    sw = min(SUB, cs + cw - ss)
                    nc.tensor.matmul(
                        ps_t[0:mh, ss - cs : ss - cs + sw],
                        lhsT=lhsT[:, ms : ms + mh],
                        rhs=rhs[:, ss : ss + sw],
                        start=True,
                        stop=True,
                    )
                return ps_t

            # ---- pass 1: max of raw out of PSUM.  DVE is the scarce engine
            # (only it can reduce), so alternate: even chunks reduce f32 from
            # PSUM directly; odd chunks get an ACT fp16 copy first and a 2x
            # fp16 DVE reduce (costs ~5e-4 relative on the max, irrelevant
            # at the 2e-2 tolerance).
            kcol = 0
            with tc.tile_pool(name="s16", bufs=3) as s16p:
                for ms, mh in M_TILES:
                    for cs, cw in CHUNKS:
                        ps_t = emit_mms(ms, mh, cs, cw, "ps1")
                        if kcol % 2 == 0:
                            nc.vector.tensor_reduce(
                                out=racc[0:mh, kcol : kcol + 1],
                                in_=ps_t[0:mh, 0:cw],
                                axis=mybir.AxisListType.X,
                                op=AX.max,
                            )
                        else:
                            sc16 = s16p.tile([128, 1024], FP16, name="sc16",
                                             tag="sc16")
                            nc.scalar.activation(sc16[0:mh, 0:cw],
                                                 ps_t[0:mh, 0:cw], AFT.Copy)
                            nc.vector.tensor_reduce(
                                out=racc[0:mh, kcol : kcol + 1],
                                in_=sc16[0:mh, 0:cw],
                                axis=mybir.AxisListType.X,
                                op=AX.max,
                            )
                        kcol += 1

            # fold local max -> one scalar -> AllReduce(max) -> s = 1/max
            rmax = cst.tile([128, 1], F32, name="rmax")
            nc.vector.tensor_reduce(out=rmax[:, :], in_=racc[:, :],
                                    axis=mybir.AxisListType.X, op=AX.max)
            # partition fold via PE transpose [128,1] -> [1,128]
            ps_tr = psp.tile([128, 1024], F32, name="ps_tr", tag="ps", bufs=4)
            nc.tensor.transpose(ps_tr[0:1, 0:128], rmax[:, :], ident[:, :])
            gmaxl = cst.tile([1, 1], F32, name="gmaxl")
            nc.vector.tensor_reduce(out=gmaxl[:, :], in_=ps_tr[0:1, 0:128],
                                    axis=mybir.AxisListType.X, op=AX.max)
            nc.sync.dma_start(cc_in[0:1, 0:1], gmaxl[:, :])
            nc.gpsimd.collective_compute(
                "AllReduce",
                AX.max,
                replica_groups=groups,
                ins=[cc_in.opt()],
                outs=[cc_out.opt()],
            )
            gsb = cst.tile([1, 1], F32, name="gsb")
            nc.sync.dma_start(gsb[:, :], cc_out[0:1, 0:1])
            ones_col = cst.tile([1, 128], F32, name="ones_col")
            nc.vector.memset(ones_col[:, :], 1.0)
            gbc = cst.tile([128, 1], F32, name="gbc")
            s_sb = cst.tile([128, 1], F32, name="s_sb")

            # ---- pass 2: recompute, scale by 1/max during the PSUM drain,
            # stream out in half-row-block DMAs on rotating queues.
            # The first 3 chunks' matmuls are issued BEFORE the 1/max
            # broadcast so the PE keeps streaming through the AllReduce
            # window (the 4-deep PSUM rotation bounds how far it can run).
            oeng = [nc.sync, nc.gpsimd]

            def emit_drain(ms, mh, cs, cw, ps_t, k2, ob):
                if k2 % 2 == 0:
                    nc.scalar.activation(
                        ob[0:mh, cs : cs + cw],
                        ps_t[0:mh, 0:cw],
                        AFT.Copy,
                        scale=s_sb[0:mh, 0:1],
                    )
                else:
                    nc.vector.tensor_scalar(
                        out=ob[0:mh, cs : cs + cw],
                        in0=ps_t[0:mh, 0:cw],
                        scalar1=s_sb[0:mh, 0:1],
                        scalar2=None,
                        op0=AX.mult,
                    )

            k2 = 0
            for i, (ms, mh) in enumerate(M_TILES):
                ob = obig.tile([128, N], F32, name="ob", tag="ob")
                if i == 0:
                    pre = [emit_mms(ms, mh, cs, cw, "ps2")
                           for cs, cw in CHUNKS[0:3]]
                    # broadcast [1,1] -> [128,1] via ones-matmul; queued on
                    # the PE after the prefill, so it runs as soon as the
                    # AllReduce lands without blocking those matmuls
                    ps_bc = psp.tile([128, 1024], F32, name="ps_bc", tag="ps",
                                     bufs=4)
                    nc.tensor.matmul(ps_bc[0:128, 0:1], lhsT=ones_col[0:1, :],
                                     rhs=gsb[0:1, 0:1], start=True, stop=True)
                    nc.vector.tensor_copy(gbc[:, :], ps_bc[0:128, 0:1])
                    nc.vector.reciprocal(s_sb[:, :], gbc[:, :])
                    for j, (cs, cw) in enumerate(CHUNKS[0:3]):
                        emit_drain(ms, mh, cs, cw, pre[j], k2, ob)
                        k2 += 1
                        if j == 0:
                            oeng[0].dma_start(
                                out[ms : ms + mh, 0:1024], ob[0:mh, 0:1024]
                            )
                        elif j == 2:
                            oeng[1].dma_start(
                                out[ms : ms + mh, 1024:3072],
                                ob[0:mh, 1024:3072],
                            )
                    rest = CHUNKS[3:]
                else:
                    rest = CHUNKS
                for j, (cs, cw) in enumerate(rest):
                    ps_t = emit_mms(ms, mh, cs, cw, "ps2")
                    emit_drain(ms, mh, cs, cw, ps_t, k2, ob)
                    k2 += 1
                    if i > 0 and j == 2:
                        oeng[i % 2].dma_start(
                            out[ms : ms + mh, 0:3072], ob[0:mh, 0:3072]
                        )
                oeng[(i + 1) % 2].dma_start(
                    out[ms : ms + mh, 3072:N], ob[0:mh, 3072:N]
                )

    _split_sync_waits(nc)
    return nc


_NC_CACHE = []


def kernel(batch: np.ndarray) -> np.ndarray:
    batch = np.asarray(batch, dtype=np.float32)
    assert batch.shape == (C, H, W)
    if not _NC_CACHE:
        _NC_CACHE.append(_build())
    nc = _NC_CACHE[0]
    in_maps = [
        {"x": np.ascontiguousarray(batch[:, k * H_LOC : (k + 1) * H_LOC, :])}
        for k in range(N_CORES)
    ]
    res = run_bass_kernel_spmd(nc, in_maps, core_ids=list(range(N_CORES)))
    full = np.concatenate([res.results[k]["out"] for k in range(N_CORES)], axis=0)
    return full[None].astype(np.float32)


# revision 22
# speedup vs baseline: 1.0926x; 1.0926x over previous
"""Distributed Trainium2 kernel for nn_Curating_of_attention_mask.

Math: batch (3,1280,1280) -> 6400 patches of 16x16 -> per-patch 3x3 channel
gram -> pairwise squared-distance matrix (6400,6400) -> global min/max
normalize -> (1,6400,6400).

Simplifications vs the reference:
 - (d - min)/(max - min) is invariant to positive rescaling, so constant
   divisors are dropped.
 - dist[i,i] = 0 and dist >= 0, so the global min is 0 (up to ~1e-7 fp
   noise): only the max must be computed/reduced; out = raw / max.
 - Grams are centered by the expected gram of unit-variance noise (256 on
   the diagonal), shrinking magnitudes ~30x and every rounding error.
 - With the 6 unique centered gram entries m and q = sum(diag^2)+2*sum(off^2),
   v = [m/16, 1, q/256] and u = [-2*m_d/16, -4*m_o/16, q/256, 1] give
   raw[i,j] = u_i . v_j = (q_i + q_j - 2<gram_i,gram_j>)/256 >= 0.
   The /16 prescale keeps everything in fp16 range so the big matmuls run
   at 1 cycle/row.

Layout: input slab loads CONTIGUOUSLY as row-major [row, col] tiles
([128,1280] + [32,1280] per channel); per-patch pixel sums happen as
(a) elementwise products + grouped 16-column reduces on ACT/GpSimd/DVE,
(b) a block-ones matmul on the PE to sum the 16 rows of each patch row.

Sharding: patch dim across 8 cores (core k owns image rows [160k,160k+160)
= patches [800k,800k+800)).  Each core builds u/v for its 800 patches,
AllGathers v (fp16), computes its [800,6400] slice of raw twice (pass 1
reduces the max straight out of PSUM, pass 2 applies 1/max during the
PSUM->SBUF drain and streams the result to HBM).  A dummy AllGather at
t=0 warms the collective stream so the real gather isn't serialized
behind the first-collective barrier.
"""

import numpy as np

import concourse.bass as bass
import concourse.bass_isa as bass_isa
import concourse.mybir as mybir
import concourse.tile as tile
from concourse.bass_utils import run_bass_kernel_spmd

F32 = mybir.dt.float32
FP16 = mybir.dt.float16
AX = mybir.AluOpType
AFT = mybir.ActivationFunctionType

N_CORES = 8
C, H, W = 3, 1280, 1280
PS = 16
HP, WP = H // PS, W // PS            # 80, 80
N = HP * WP                          # 6400
H_LOC = H // N_CORES                 # 160 image rows per core
TP = H_LOC // PS                     # 10 patch-rows per core
N_LOC = TP * WP                      # 800 patches per core
K = 8                                # feature dim of u/v
NEG_BIG = -3.0e38
CENTER = 256.0

# feature order: diagonals first, then off-diagonals
PAIRS = [(0, 0), (1, 1), (2, 2), (0, 1), (0, 2), (1, 2)]

# output row tiles and column chunks (each chunk = one PSUM tile; matmuls
# are issued in <=512-column sub-tiles because a PSUM bank is 512 f32)
M_TILES = [(ms, min(128, N_LOC - ms)) for ms in range(0, N_LOC, 128)]     # 7
CHUNKS = [(cs, min(1024, N - cs)) for cs in range(0, N, 1024)]            # 7
SUB = 512

# phase-A input column chunks
ACH = [(0, 640), (640, 640)]

# walrus in this container accepts at most 1 sync-wait command per
# instruction; Tile's tail drain can carry several.  Split extras onto
# preceding NOPs on the same engine (stream order preserves semantics).
_MAX_WAITS = 1


def _split_sync_waits(nc):
    n_fixed = 0
    for func in nc.m.functions:
        for bb in func.blocks:
            new_insts = []
            for inst in bb.instructions:
                si = inst.sync_info
                if si is not None and si.on_wait and len(si.on_wait) > _MAX_WAITS:
                    waits = list(si.on_wait)
                    keep = waits[-_MAX_WAITS:]
                    extra = waits[:-_MAX_WAITS]
                    chunks = [
                        extra[i : i + _MAX_WAITS]
                        for i in range(0, len(extra), _MAX_WAITS)
                    ]
                    for ci, chunk in enumerate(chunks):
                        nop = mybir.InstNoOp(
                            name=f"{inst.name}-waitsplit-{ci}",
                            engine=inst.engine,
                            ins=[],
                            outs=[],
                            sync_info=mybir.SyncInfo(on_wait=chunk, on_update=[]),
                        )
                        new_insts.append(nop)
                        n_fixed += 1
                    si.on_wait = keep
                new_insts.append(inst)
            bb.instructions[:] = new_insts
    return n_fixed


def _build():
    nc = bass.Bass(num_devices=N_CORES)
    x = nc.dram_tensor("x", [C, H_LOC, W], F32, kind="ExternalInput")
    out = nc.dram_tensor("out", [N_LOC, N], F32, kind="ExternalOutput")
    groups = [list(range(N_CORES))]

    with tile.TileContext(nc, num_cores=N_CORES) as tc:
        with (
            tc.tile_pool(name="dram", bufs=1, space="DRAM") as dpool,
            tc.tile_pool(name="cst", bufs=1) as cst,
            tc.tile_pool(name="obig", bufs=2) as obig,
            tc.tile_pool(name="ps", bufs=1, space="PSUM") as psp,
        ):
            v_dram = dpool.tile([K, N_LOC], FP16, name="v_dram")
            u_dram = dpool.tile([K, N_LOC], FP16, name="u_dram")
            vall = dpool.tile([K * N_CORES, N_LOC], FP16, addr_space="Shared",
                              name="vall")
            cc_wo = dpool.tile([1, 8 * N_CORES], F32, addr_space="Shared",
                               name="cc_wo")
            cc_in = dpool.tile([1, 8], F32, name="cc_in")
            cc_out = dpool.tile([1, 8], F32, addr_space="Shared", name="cc_out")

            # ---- warm the collective stream: dummy 32-byte gather at t=0.
            # The CC cores boot ~21us into the NEFF and the first collective
            # pays a ~30us barrier plus a ~15us first-op penalty; paying all
            # of that on a dummy while phase A runs keeps the real gather at
            # its ~5.5us steady-state cost.
            cc_wi = dpool.tile([1, 8], F32, name="cc_wi")
            w8 = cst.tile([1, 8], F32, name="w8")
            nc.vector.memset(w8[:, :], 0.0)
            nc.sync.dma_start(cc_wi[:, :], w8[:, :])
            nc.gpsimd.collective_compute(
                "AllGather",
                AX.bypass,
                replica_groups=groups,
                ins=[cc_wi.opt()],
                outs=[cc_wo.opt()],
            )

            # constants: patch-row summing matrices (block-ones)
            # S[r, t] = 1 iff r//16 == t, built as 0 <= (r - 16t) < 16
            def block_ones(name, nparts, ncols):
                d = cst.tile([nparts, ncols], F32, name=f"{name}_d")
                nc.gpsimd.iota(d[:, :], pattern=[[-16, ncols]], base=0,
                               channel_multiplier=1,
                               allow_small_or_imprecise_dtypes=True)
                ge = cst.tile([nparts, ncols], F32, name=f"{name}_ge")
                nc.vector.tensor_scalar(out=ge[:, :], in0=d[:, :],
                                        scalar1=0.0, scalar2=None,
                                        op0=AX.is_ge)
                s = cst.tile([nparts, ncols], F32, name=name)
                nc.vector.tensor_scalar(out=s[:, :], in0=d[:, :],
                                        scalar1=16.0, scalar2=None,
                                        op0=AX.is_lt)
                nc.vector.tensor_mul(s[:, :], s[:, :], ge[:, :])
                return s

            S1 = block_ones("S1", 128, K)
            S2 = block_ones("S2", 32, 2)

            racc = cst.tile([128, 56], F32, name="racc")
            nc.vector.memset(racc[:, :], NEG_BIG)
            sc8 = cst.tile([1, 8], F32, name="sc8")
            nc.vector.memset(sc8[:, :], NEG_BIG)
            # pre-stage the AllReduce input padding early (off critical path)
            nc.sync.dma_start(cc_in[:, :], sc8[:, :])

            # identity matrix for the PE transpose in the max fold
            iota2d = cst.tile([128, 128], F32, name="iota2d")
            nc.gpsimd.iota(iota2d[:, :], pattern=[[1, 128]], base=0,
                           channel_multiplier=0,
                           allow_small_or_imprecise_dtypes=True)
            iota_col = cst.tile([128, 1], F32, name="iota_col")
            nc.gpsimd.iota(iota_col[:, :], pattern=[[0, 1]], base=0,
                           channel_multiplier=1,
                           allow_small_or_imprecise_dtypes=True)
            ident = cst.tile([128, 128], F32, name="ident")
            nc.vector.tensor_scalar(
                out=ident[:, :], in0=iota2d[:, :], scalar1=iota_col[:, 0:1],
                scalar2=None, op0=AX.is_equal,
            )

            # ---- phase A: contiguous row-major loads + per-patch grams ----
            # feature-major pcol layout [p, f*80+w] so reduce writes are
            # contiguous (strided writes halve DVE throughput)
            pcol6a = cst.tile([128, 6 * WP], F32, name="pcol6a")
            pcol6b = cst.tile([32, 6 * WP], F32, name="pcol6b")
            pca_r = pcol6a.rearrange("p (f w) -> p f w", w=WP)
            pcb_r = pcol6b.rearrange("p (f w) -> p f w", w=WP)

            with (
                tc.tile_pool(name="xsl", bufs=1) as xsl,
                tc.tile_pool(name="prodp", bufs=3) as prodp,
            ):
                xa = [xsl.tile([128, W], F32, name=f"xa{c}") for c in range(C)]
                xb = [xsl.tile([32, W], F32, name=f"xb{c}") for c in range(C)]
                dmae = [nc.sync, nc.scalar, nc.gpsimd]
                for h, (hs, hw) in enumerate(ACH):
                    sl = slice(hs, hs + hw)
                    for c in range(C):
                        dmae[c].dma_start(xa[c][:, sl], x[c, 0:128, sl])
                        dmae[c].dma_start(xb[c][:, sl], x[c, 128:160, sl])
                    wsl = slice(hs // PS, (hs + hw) // PS)
                    for f, (a, b) in enumerate(PAIRS):
                        pa = prodp.tile([128, 640], F32, name="pa", tag="pa")
                        pb = prodp.tile([32, 640], F32, name="pb", tag="pb")
                        if a == b:
                            nc.scalar.activation(pa[:, :], xa[a][:, sl],
                                                 AFT.Square)
                            nc.scalar.activation(pb[:, :], xb[a][:, sl],
                                                 AFT.Square)
                        else:
                            nc.vector.tensor_mul(pa[:, :], xa[a][:, sl],
                                                 xa[b][:, sl])
                            nc.vector.tensor_mul(pb[:, :], xb[a][:, sl],
                                                 xb[b][:, sl])
                        nc.vector.tensor_reduce(
                            out=pca_r[:, f, wsl],
                            in_=pa.rearrange("p (w b) -> p w b", b=PS),
                            axis=mybir.AxisListType.X,
                            op=AX.add,
                        )
                        nc.vector.tensor_reduce(
                            out=pcb_r[:, f, wsl],
                            in_=pb.rearrange("p (w b) -> p w b", b=PS),
                            axis=mybir.AxisListType.X,
                            op=AX.add,
                        )

            # sum the 16 rows of each patch row on the PE
            psG = psp.tile([128, 1024], F32, name="psG", tag="ps", bufs=4)
            nc.tensor.matmul(psG[0:8, 0 : 6 * WP], lhsT=S1[:, :],
                             rhs=pcol6a[:, :], start=True, stop=True)
            nc.tensor.matmul(psG[32:34, 0 : 6 * WP], lhsT=S2[:, :],
                             rhs=pcol6b[:, :], start=True, stop=True)
            G = cst.tile([TP, 6 * WP], F32, name="G")
            nc.scalar.activation(G[0:8, :], psG[0:8, 0 : 6 * WP], AFT.Copy)
            # partitions 32:34 -> 8:10 need a rebase: ACT copy (32-aligned)
            # into SBUF, then an SBUF->SBUF DMA (DMA rebases freely)
            Gtmp = cst.tile([34, 6 * WP], F32, name="Gtmp")
            nc.scalar.activation(Gtmp[32:34, :], psG[32:34, 0 : 6 * WP],
                                 AFT.Copy)
            nc.sync.dma_start(G[8:10, :], Gtmp[32:34, :])
            G_f = G.rearrange("p (f w) -> p f w", w=WP)

            # center diagonal gram entries, then q = sum(d^2) + 2*sum(o^2)
            nc.vector.tensor_scalar_add(G_f[:, 0:3, :], G_f[:, 0:3, :], -CENTER)
            Gsq = cst.tile([TP, 6 * WP], F32, name="Gsq")
            nc.vector.tensor_mul(Gsq[:, :], G[:, :], G[:, :])
            Gsq_f = Gsq.rearrange("p (f w) -> p f w", w=WP)
            qd = cst.tile([TP, WP], F32, name="qd")
            qo = cst.tile([TP, WP], F32, name="qo")
            nc.vector.tensor_add(qd[:, :], Gsq_f[:, 0, :], Gsq_f[:, 1, :])
            nc.vector.tensor_add(qd[:, :], qd[:, :], Gsq_f[:, 2, :])
            nc.vector.tensor_add(qo[:, :], Gsq_f[:, 3, :], Gsq_f[:, 4, :])
            nc.vector.tensor_add(qo[:, :], qo[:, :], Gsq_f[:, 5, :])
            q = cst.tile([TP, WP], F32, name="q")
            nc.vector.scalar_tensor_tensor(
                out=q[:, :], in0=qo[:, :], scalar=2.0, in1=qd[:, :],
                op0=AX.mult, op1=AX.add,
            )

            # fp16 feature heaps [t, slot*80+w]; slots 0-5 gram, 6/7 one & q
            vheap = cst.tile([TP, K * WP], FP16, name="vheap")
            uheap = cst.tile([TP, K * WP], FP16, name="uheap")
            vh_r = vheap.rearrange("p (s w) -> p s w", w=WP)
            uh_r = uheap.rearrange("p (s w) -> p s w", w=WP)
            nc.scalar.activation(vh_r[:, 0:6, :], G_f[:, 0:6, :], AFT.Copy,
                                 scale=1.0 / 16.0)
            nc.vector.memset(vheap[:, 6 * WP : 7 * WP], 1.0)
            nc.scalar.activation(vheap[:, 7 * WP : 8 * WP], q[:, :], AFT.Copy,
                                 scale=1.0 / 256.0)
            nc.sync.dma_start(
                v_dram.rearrange("s (t w) -> t s w", w=WP), vh_r[:, :, :]
            )

            # ---- all-gather v across cores (fp16) ----
            nc.gpsimd.collective_compute(
                "AllGather",
                AX.bypass,
                replica_groups=groups,
                ins=[v_dram.opt()],
                outs=[vall.opt()],
            )

            nc.scalar.activation(uh_r[:, 0:3, :], G_f[:, 0:3, :], AFT.Copy,
                                 scale=-2.0 / 16.0)
            nc.scalar.activation(uh_r[:, 3:6, :], G_f[:, 3:6, :], AFT.Copy,
                                 scale=-4.0 / 16.0)
            nc.scalar.activation(uheap[:, 6 * WP : 7 * WP], q[:, :], AFT.Copy,
                                 scale=1.0 / 256.0)
            nc.vector.memset(uheap[:, 7 * WP : 8 * WP], 1.0)
            nc.scalar.dma_start(
                u_dram.rearrange("s (t w) -> t s w", w=WP), uh_r[:, :, :]
            )
            lhsT = cst.tile([K, N_LOC], FP16, name="lhsT")
            nc.scalar.dma_start(lhsT[:, :], u_dram[:, :])

            rhs = cst.tile([K, N], FP16, name="rhs")
            nc.sync.dma_start(
                rhs.rearrange("f (r l) -> f r l", l=N_LOC),
                vall.rearrange("(r f) l -> f r l", f=K),
            )

            def emit_mms(ms, mh, cs, cw, name):
                ps_t = psp.tile([128, 1024], F32, name=name, tag="ps", bufs=4)
                for ss in range(cs, cs + cw, SUB):
                    sw = min(SUB, cs + cw - ss)
                    nc.tensor.matmul(
                        ps_t[0:mh, ss - cs : ss - cs + sw],
                        lhsT=lhsT[:, ms : ms + mh],
                        rhs=rhs[:, ss : ss + sw],
                        start=True,
                        stop=True,
                    )
                return ps_t

            # ---- pass 1: max of raw straight out of PSUM ----
            kcol = 0
            for ms, mh in M_TILES:
                for cs, cw in CHUNKS:
                    ps_t = emit_mms(ms, mh, cs, cw, "ps1")
                    nc.vector.tensor_reduce(
                        out=racc[0:mh, kcol : kcol + 1],
                        in_=ps_t[0:mh, 0:cw],
                        axis=mybir.AxisListType.X,
                        op=AX.max,
                    )
                    kcol += 1

            # fold local max -> one scalar -> AllReduce(max) -> s = 1/max
            rmax = cst.tile([128, 1], F32, name="rmax")
            nc.vector.tensor_reduce(out=rmax[:, :], in_=racc[:, :],
                                    axis=mybir.AxisListType.X, op=AX.max)
            # partition fold via PE transpose [128,1] -> [1,128]
            ps_tr = psp.tile([128, 1024], F32, name="ps_tr", tag="ps", bufs=4)
            nc.tensor.transpose(ps_tr[0:1, 0:128], rmax[:, :], ident[:, :])
            gmaxl = cst.tile([1, 1], F32, name="gmaxl")
            nc.vector.tensor_reduce(out=gmaxl[:, :], in_=ps_tr[0:1, 0:128],
                                    axis=mybir.AxisListType.X, op=AX.max)
            nc.sync.dma_start(cc_in[0:1, 0:1], gmaxl[:, :])
            nc.gpsimd.collective_compute(
                "AllReduce",
                AX.max,
                replica_groups=groups,
                ins=[cc_in.opt()],
                outs=[cc_out.opt()],
            )
            gsb = cst.tile([1, 1], F32, name="gsb")
            nc.sync.dma_start(gsb[:, :], cc_out[0:1, 0:1])
            ones_col = cst.tile([1, 128], F32, name="ones_col")
            nc.vector.memset(ones_col[:, :], 1.0)
            gbc = cst.tile([128, 1], F32, name="gbc")
            s_sb = cst.tile([128, 1], F32, name="s_sb")

            # ---- pass 2: recompute, scale by 1/max during the PSUM drain,
            # stream out in half-row-block DMAs on rotating queues.
            # The first 3 chunks' matmuls are issued BEFORE the 1/max
            # broadcast so the PE keeps streaming through the AllReduce
            # window (the 4-deep PSUM rotation bounds how far it can run).
            oeng = [nc.sync, nc.gpsimd]

            def emit_drain(ms, mh, cs, cw, ps_t, k2, ob):
                if k2 % 2 == 0:
                    nc.scalar.activation(
                        ob[0:mh, cs : cs + cw],
                        ps_t[0:mh, 0:cw],
                        AFT.Copy,
                        scale=s_sb[0:mh, 0:1],
                    )
                else:
                    nc.vector.tensor_scalar(
                        out=ob[0:mh, cs : cs + cw],
                        in0=ps_t[0:mh, 0:cw],
                        scalar1=s_sb[0:mh, 0:1],
                        scalar2=None,
                        op0=AX.mult,
                    )

            k2 = 0
            for i, (ms, mh) in enumerate(M_TILES):
                ob = obig.tile([128, N], F32, name="ob", tag="ob")
                if i == 0:
                    pre = [emit_mms(ms, mh, cs, cw, "ps2")
                           for cs, cw in CHUNKS[0:3]]
                    # broadcast [1,1] -> [128,1] via ones-matmul; queued on
                    # the PE after the prefill, so it runs as soon as the
                    # AllReduce lands without blocking those matmuls
                    ps_bc = psp.tile([128, 1024], F32, name="ps_bc", tag="ps",
                                     bufs=4)
                    nc.tensor.matmul(ps_bc[0:128, 0:1], lhsT=ones_col[0:1, :],
                                     rhs=gsb[0:1, 0:1], start=True, stop=True)
                    nc.vector.tensor_copy(gbc[:, :], ps_bc[0:128, 0:1])
                    nc.vector.reciprocal(s_sb[:, :], gbc[:, :])
                    for j, (cs, cw) in enumerate(CHUNKS[0:3]):
                        emit_drain(ms, mh, cs, cw, pre[j], k2, ob)
                        k2 += 1
                        if j == 0:
                            oeng[0].dma_start(
                                out[ms : ms + mh, 0:1024], ob[0:mh, 0:1024]
                            )
                        elif j == 2:
                            oeng[1].dma_start(
                                out[ms : ms + mh, 1024:3072],
                                ob[0:mh, 1024:3072],
                            )
                    rest = CHUNKS[3:]
                else:
                    rest = CHUNKS
                for j, (cs, cw) in enumerate(rest):
                    ps_t = emit_mms(ms, mh, cs, cw, "ps2")
                    emit_drain(ms, mh, cs, cw, ps_t, k2, ob)
                    k2 += 1
                    if i > 0 and j == 2:
                        oeng[i % 2].dma_start(
                            out[ms : ms + mh, 0:3072], ob[0:mh, 0:3072]
                        )
                oeng[(i + 1) % 2].dma_start(
                    out[ms : ms + mh, 3072:N], ob[0:mh, 3072:N]
                )

    _split_sync_waits(nc)
    return nc


_NC_CACHE = []


def kernel(batch: np.ndarray) -> np.ndarray:
    batch = np.asarray(batch, dtype=np.float32)
    assert batch.shape == (C, H, W)
    if not _NC_CACHE:
        _NC_CACHE.append(_build())
    nc = _NC_CACHE[0]
    in_maps = [
        {"x": np.ascontiguousarray(batch[:, k * H_LOC : (k + 1) * H_LOC, :])}
        for k in range(N_CORES)
    ]
    res = run_bass_kernel_spmd(nc, in_maps, core_ids=list(range(N_CORES)))
    full = np.concatenate([res.results[k]["out"] for k in range(N_CORES)], axis=0)
    return full[None].astype(np.float32)
